# revision 11
# baseline (speedup 1.0000x reference)
"""Trainium2 Bass kernel for sliding-window multi-head attention.

Problem (nn_MultiHeadAttention_74285754352148):
  B=2, S=2048, D=1024, H=16, HD=64, WINDOW=512 (causal, j in [i-256, i]),
  RoPE theta=10000, out = softmax(mask(QK^T)/8) V @ Wo + bo.

Sharding: batch x sequence across 8 cores (core c: batch c//4, tokens
[512*(c%4), 512*(c%4)+512)). Each core recomputes K/V for a 256-token halo;
no collectives.

v2 design (vs f32r baseline):
  - all matmul operands bf16 (halves HBM traffic, enables FWL weight loads;
    fp32 PSUM accumulation throughout, max rel err ~4e-3 vs 2e-2 budget)
  - scores per 256-query block cover 768 key-cols (128/256/256/128 per key
    chunk) instead of 1024: -25% exp/mask/score work; the two heads of a
    pair run in PE row groups 0-63/64-127, interleaved per chunk so they
    genuinely co-run
  - denominator via validity-masked ones column of V (replaces corr table)
  - engine split: rope mults/adds + softmax normalize on DVE, exp + copies
    + Wo bias on ACT, band masks on Pool (Pool cannot access PSUM on TRN2)
  - DMA order: xt query cols + Wq first (tables on the scalar DMA queue) so
    Q-proj starts ~6.5us in instead of ~17us
  - PE order interleaves V-projection chunks between score groups, and the
    qp0/qp1 transposes around the qp1 context groups, so the softmax chain
    (ACT exp -> Pool mask -> DVE normalize) never stalls the PE
"""

import numpy as np
import ml_dtypes

import concourse.bass as bass
import concourse.bacc as bacc
import concourse.mybir as mybir
from concourse.tile import TileContext
from concourse.bass import ts
from concourse.bass_utils import run_bass_kernel_spmd

F32 = mybir.dt.float32
BF16 = mybir.dt.bfloat16

B, S, D = 2, 2048, 1024
H, HD = 16, 64
HALF_W = 256          # window // 2: query i attends keys [i-256, i]
TC = 512              # tokens per core
TH = TC + HALF_W      # tokens incl halo = 768
DC = D // 128         # 8 partition chunks of the model dim
NTC = TH // 128       # token chunks incl halo = 6
VW = HD + 1           # per-head V width incl ones column = 65
THETA = 10000.0

BYPASS = mybir.AluOpType.bypass
MULT = mybir.AluOpType.mult


def build_nc(loop_repeat=None):
    nc = bacc.Bacc(None, target_bir_lowering=False)

    xt = nc.dram_tensor("xt", [D, TH], BF16, kind="ExternalInput")
    wq = nc.dram_tensor("wq", [128, DC * D], BF16, kind="ExternalInput")
    wk = nc.dram_tensor("wk", [128, DC * D], BF16, kind="ExternalInput")
    wv = nc.dram_tensor("wv", [D, D], BF16, kind="ExternalInput")
    wo = nc.dram_tensor("wo", [128, DC * D], BF16, kind="ExternalInput")
    bo = nc.dram_tensor("bo", [128, DC], F32, kind="ExternalInput")
    cosk = nc.dram_tensor("cosk", [128, TH], BF16, kind="ExternalInput")
    sink2 = nc.dram_tensor("sink2", [128, TH], BF16, kind="ExternalInput")
    vtbl = nc.dram_tensor("vtbl", [128, NTC * H], BF16, kind="ExternalInput")
    ident_d = nc.dram_tensor("ident", [128, 128], BF16, kind="ExternalInput")
    perm_d = nc.dram_tensor("perm32", [128, 128], BF16, kind="ExternalInput")
    outT = nc.dram_tensor("outT", [D, TC], F32, kind="ExternalOutput")

    with TileContext(nc) as tc:
        with (
            tc.tile_pool(name="tbl", bufs=1) as tbl,
            tc.tile_pool(name="xtp", bufs=2) as xtp,
            tc.tile_pool(name="wpool", bufs=4) as wpool,
            tc.tile_pool(name="qkp", bufs=1) as qkp,
            tc.tile_pool(name="vp", bufs=1) as vp,
            tc.tile_pool(name="uwp", bufs=3) as uwp,
            tc.tile_pool(name="pp", bufs=24) as pp,
            tc.tile_pool(name="sm", bufs=8) as sm,
            tc.tile_pool(name="cxp", bufs=4) as cxp,
            tc.tile_pool(name="cxtp", bufs=1) as cxtp,
            tc.tile_pool(name="op", bufs=3) as op,
            tc.tile_pool(name="proj_ps", bufs=2, space="PSUM") as proj_ps,
            tc.tile_pool(name="sc_ps", bufs=2, space="PSUM") as sc_ps,
            tc.tile_pool(name="ctx_ps", bufs=2, space="PSUM") as ctx_ps,
        ):
            # ---- constant/table loads (iteration-invariant) ----
            cosk_sb = tbl.tile([128, TH], BF16)
            sink2_sb = tbl.tile([128, TH], BF16)
            vtbl_sb = tbl.tile([128, NTC * H], BF16)
            bo_sb = tbl.tile([128, DC], F32)
            ident = tbl.tile([128, 128], BF16)
            perm32 = tbl.tile([128, 128], BF16)
            for t_dram, t_sb in [
                (cosk, cosk_sb),
                (sink2, sink2_sb),
                (vtbl, vtbl_sb),
                (bo, bo_sb),
                (ident_d, ident),
                (perm_d, perm32),
            ]:
                # scalar-engine DMA queue: runs in parallel with the sync
                # queue so table loads don't delay xt/Wq at startup
                nc.scalar.dma_start(out=t_sb, in_=t_dram[:, :])

            def body():
                # ---- input loads: query cols + Wq first so Q-proj starts early
                xt_sb = xtp.tile([128, DC, TH], BF16)
                for k in range(DC):
                    nc.sync.dma_start(
                        out=xt_sb[:, k, HALF_W:TH], in_=xt[ts(k, 128), HALF_W:TH]
                    )

                def load_w_blocked(w_dram, nm):
                    """dc-blocked: host layout [p, dc, k, c]; access (k, dc)."""
                    halves = []
                    for hh in range(2):
                        w_sb = wpool.tile(
                            [128, DC // 2, DC, 128], BF16, tag="w", name=f"w_{nm}{hh}"
                        )
                        for dcl in range(DC // 2):
                            off = (hh * 4 + dcl) * D
                            nc.sync.dma_start(
                                out=w_sb[:, dcl], in_=w_dram[:, off : off + D]
                            )
                        halves.append(w_sb)
                    return lambda k, dc: halves[dc // 4][:, dc % 4, k]

                def load_w(w_dram, nm):
                    """Two half-matrix tiles [128, 4, 1024] sharing wpool slots."""
                    halves = []
                    for hh in range(2):
                        w_sb = wpool.tile(
                            [128, DC // 2, D], BF16, tag="w", name=f"w_{nm}{hh}"
                        )
                        for k in range(DC // 2):
                            nc.sync.dma_start(
                                out=w_sb[:, k], in_=w_dram[ts(hh * 4 + k, 128), :]
                            )
                        halves.append(w_sb)
                    return lambda k: halves[k // 4][:, k % 4]

                wq_at = load_w_blocked(wq, "q")
                wk_at = load_w_blocked(wk, "k")
                for k in range(DC):
                    nc.sync.dma_start(
                        out=xt_sb[:, k, 0:HALF_W], in_=xt[ts(k, 128), 0:HALF_W]
                    )

                qrope = qkp.tile([128, DC, TC], BF16)
                krope = qkp.tile([128, DC, TH], BF16)

                # RoPE epilogue, software-pipelined by one projection group:
                # u = ps*cos, w = ps*sin2 on DVE (Pool cannot touch PSUM and
                # supports no fast multiply); the PE shift matmul + DVE add
                # are deferred one group so the PE never waits on the DVE.
                pending = []

                def rope_start(ps, cslc, out_ap, n):
                    u = uwp.tile([128, n], BF16, tag="u")
                    nc.vector.scalar_tensor_tensor(
                        out=u, in0=ps, scalar=1.0, in1=cosk_sb[:, cslc],
                        op0=BYPASS, op1=MULT,
                    )
                    w = uwp.tile([128, n], BF16, tag="w")
                    nc.vector.scalar_tensor_tensor(
                        out=w, in0=ps, scalar=1.0, in1=sink2_sb[:, cslc],
                        op0=BYPASS, op1=MULT,
                    )
                    pending.append((u, w, out_ap, n))

                def rope_flush():
                    if not pending:
                        return
                    u, w, out_ap, n = pending.pop(0)
                    ws_ps = sc_ps.tile([128, n], F32, tag="sc")
                    nc.tensor.matmul(ws_ps, perm32, w, start=True, stop=True)
                    nc.vector.tensor_add(out_ap, ws_ps, u)

                # ---- Q^T projection + RoPE (dim-major) ----
                for dc in range(DC):
                    ps = proj_ps.tile([128, TC], F32, tag="proj")
                    for k in range(DC):
                        nc.tensor.matmul(
                            ps, wq_at(k, dc), xt_sb[:, k, HALF_W:TH],
                            start=(k == 0), stop=(k == DC - 1),
                        )
                    rope_flush()
                    rope_start(ps, slice(HALF_W, TH), qrope[:, dc], TC)

                # ---- K^T projection + RoPE, two 384-col halves ----
                for dc in range(DC):
                    for half in range(2):
                        cs = slice(half * 384, half * 384 + 384)
                        ps = proj_ps.tile([128, 384], F32, tag="proj")
                        for k in range(DC):
                            nc.tensor.matmul(
                                ps, wk_at(k, dc), xt_sb[:, k, cs],
                                start=(k == 0), stop=(k == DC - 1),
                            )
                        rope_flush()
                        rope_start(ps, cs, krope[:, dc, cs], 384)
                rope_flush()
                rope_flush()

                wv_at = load_w(wv, "v")
                wo_at = load_w_blocked(wo, "o")

                # ---- V tile: token-major, 65-wide per-head groups; col 64
                # holds the token-validity indicator (denominator counts only
                # real tokens; zero-padded halo tokens contribute 0).
                v_sb = vp.tile([128, NTC, H * VW], BF16)
                for tcn in range(NTC):
                    nc.vector.tensor_copy(
                        v_sb[:, tcn].rearrange("p (h c) -> p h c", c=VW)[:, :, HD:VW],
                        vtbl_sb[:, tcn * H : tcn * H + H].rearrange(
                            "p (a b) -> p a b", b=1
                        ),
                    )

                def v_chunk(tcn, half):
                    """V projection for one 128-token chunk, 512-dim half."""
                    ps = proj_ps.tile([128, 512], F32, tag="proj")
                    for k in range(DC):
                        nc.tensor.matmul(
                            ps, xt_sb[:, k, ts(tcn, 128)], wv_at(k)[:, ts(half, 512)],
                            start=(k == 0), stop=(k == DC - 1),
                        )
                    v_grp = v_sb[:, tcn].rearrange("p (h c) -> p h c", c=VW)
                    nc.scalar.copy(
                        out=v_grp[:, half * 8 : half * 8 + 8, 0:HD],
                        in_=ps.rearrange("p (h c) -> p h c", c=HD),
                    )

                # ---- attention ----
                # Per 256-query block qp and head pair h2: psS [128, 2, 512]
                # holds 4 key chunks at (region, cols): kc0 -> (0, 0:128),
                # kc1 -> (0, 128:384), kc2 -> (1, 0:256), kc3 -> (1, 256:384).
                # Queries covered: kc0 [0:128), kc1/kc2 [0:256), kc3 [128:256).
                # The two heads of a pair run in PE row groups 0-63 / 64-127
                # (genuinely concurrent on HW), interleaved per chunk.
                KCN = [128, 256, 256, 128]          # score cols per chunk
                KCQ = [0, 0, 0, 128]                # first query col per chunk
                KCPOS = [(0, 0), (0, 128), (1, 0), (1, 256)]  # psS (region, col)
                AFF = [
                    (1, 0, [[-1, 128]]),     # kc0: x - y >= 0
                    (1, 128, [[-1, 256]]),   # kc1: x - y + 128 >= 0
                    (-1, 0, [[1, 256]]),     # kc2: y - x >= 0
                    (-1, 0, [[1, 128]]),     # kc3 (local y): y - x >= 0
                ]

                def scores_group(qp, h2):
                    """Scores + exp for both heads of pair h2, query block qp.
                    Returns the two masked pT tiles [128, 2, 384]."""
                    psS = [
                        sc_ps.tile([128, 2, 512], F32, tag="sc",
                                   name=f"psS_{qp}_{h2}_{i}")
                        for i in range(2)
                    ]
                    for kc in range(4):
                        kcol = qp * 256 + kc * 128
                        qcol = qp * 256 + KCQ[kc]
                        reg, col = KCPOS[kc]
                        for hp_i in range(2):
                            hp = 64 * hp_i
                            nc.tensor.matmul(
                                psS[hp_i][:, reg, col : col + KCN[kc]],
                                krope[hp : hp + 64, h2, kcol : kcol + 128],
                                qrope[hp : hp + 64, h2, qcol : qcol + KCN[kc]],
                                start=True, stop=True,
                            )
                    pTs = []
                    for hp_i in range(2):
                        pT = pp.tile([128, 2, 384], BF16, tag="pT",
                                     name=f"pT_{qp}_{h2}_{hp_i}")
                        nc.scalar.activation(
                            pT, psS[hp_i][:, :, 0:384],
                            mybir.ActivationFunctionType.Exp, scale=0.125,
                        )
                        for kc in range(4):
                            reg, col = KCPOS[kc]
                            cm, base, pat = AFF[kc]
                            nc.gpsimd.affine_select(
                                out=pT[:, reg, col : col + KCN[kc]],
                                in_=pT[:, reg, col : col + KCN[kc]],
                                compare_op=mybir.AluOpType.is_ge, fill=0.0,
                                base=base, channel_multiplier=cm, pattern=pat,
                            )
                        pTs.append(pT)
                    return pTs

                # pT chunk slices for ctx: (region, col, width) per (hf, i)
                CTX_SL = [
                    [(0, 0, 128), (0, 128, 128), (1, 0, 128)],      # hf=0: kc0,kc1,kc2
                    [(0, 256, 128), (1, 128, 128), (1, 256, 128)],  # hf=1: kc1,kc2,kc3
                ]

                def ctx_group(qp, h2, pTs, ctx_hf):
                    """Context for both heads of pair h2: psC [128,2,65] per
                    head (hf regions), then normalize into ctx_hf tiles."""
                    for hp_i in range(2):
                        h = 2 * h2 + hp_i
                        pT = pTs[hp_i]
                        psC = ctx_ps.tile([128, 2, VW], F32, tag="ctx")
                        for hf in range(2):
                            for i, (reg, col, wdt) in enumerate(CTX_SL[hf]):
                                kc = [0, 1, 2][i] if hf == 0 else [1, 2, 3][i]
                                tcn = qp * 2 + kc
                                nc.tensor.matmul(
                                    psC[:, hf],
                                    pT[:, reg, col : col + wdt],
                                    v_sb[:, tcn, h * VW : h * VW + VW],
                                    start=(i == 0), stop=(i == 2),
                                )
                        rinv = sm.tile([128, 2], F32, tag="rinv")
                        nc.vector.reciprocal(rinv, psC[:, :, HD])
                        for hf in range(2):
                            nc.vector.tensor_scalar_mul(
                                ctx_hf[hf][:, h * HD : h * HD + HD],
                                psC[:, hf, 0:HD],
                                rinv[:, hf : hf + 1],
                            )

                ctxT = cxtp.tile([128, DC, TC], BF16)

                def transpose_pair(qp, dc, ctx_hf):
                    for hf in range(2):
                        qb = qp * 2 + hf
                        psT = ctx_ps.tile([128, 128], BF16, tag="ctx",
                                          name=f"psT_{qp}_{hf}_{dc}")
                        nc.tensor.transpose(psT, ctx_hf[hf][:, ts(dc, 128)], ident)
                        if hf == 0:
                            nc.vector.tensor_copy(ctxT[:, dc, ts(qb, 128)], psT)
                        else:
                            nc.scalar.copy(out=ctxT[:, dc, ts(qb, 128)], in_=psT)

                def wo_block(qpo):
                    cs = slice(qpo * 256, qpo * 256 + 256)
                    for dco in range(DC):
                        ps = sc_ps.tile([128, 256], F32, tag="sc")
                        for k in range(DC):
                            nc.tensor.matmul(
                                ps, wo_at(k, dco), ctxT[:, k, cs],
                                start=(k == 0), stop=(k == DC - 1),
                            )
                        o_sb = op.tile([128, 256], F32, tag="o")
                        nc.scalar.activation(
                            o_sb, ps, mybir.ActivationFunctionType.Identity,
                            bias=bo_sb[:, dco : dco + 1], scale=1.0,
                        )
                        nc.sync.dma_start(out=outT[ts(dco, 128), cs], in_=o_sb)

                ctx_qp = {
                    qp: [
                        cxp.tile([128, D], BF16, tag="ctx", name=f"ctx_{qp}_{i}")
                        for i in range(2)
                    ]
                    for qp in range(2)
                }

                # loop 1: scores qp0 interleaved with V chunks tcn 0-3
                pts0 = []
                for h2 in range(8):
                    pts0.append(scores_group(0, h2))
                    v_chunk(h2 // 2, h2 % 2)
                # loop 2: scores qp1 + ctx qp0 + V chunks tcn 4-5
                pts1 = []
                for h2 in range(8):
                    pts1.append(scores_group(1, h2))
                    ctx_group(0, h2, pts0[h2], ctx_qp[0])
                    if h2 < 4:
                        v_chunk(4 + h2 // 2, h2 % 2)
                # phase 3: qp0 transposes + qp1 ctx + qp1 transposes
                # (pipelined by one so the PE never waits on the DVE
                # normalize), then the two Wo blocks
                for dc in range(DC):
                    transpose_pair(0, dc, ctx_qp[0])
                    ctx_group(1, dc, pts1[dc], ctx_qp[1])
                    if dc > 0:
                        transpose_pair(1, dc - 1, ctx_qp[1])
                transpose_pair(1, DC - 1, ctx_qp[1])
                wo_block(0)
                wo_block(1)

            if loop_repeat is None:
                body()
            else:
                with tc.For_i(0, loop_repeat, 1):
                    body()

    nc.compile()
    return nc


_NC_CACHE = None


def _get_nc():
    global _NC_CACHE
    if _NC_CACHE is None:
        _NC_CACHE = build_nc()
    return _NC_CACHE


def _host_tables(positions):
    """RoPE cos/sin tables, dim-major, tiled to 128 partitions (2 heads)."""
    inv_freq = 1.0 / (THETA ** (np.arange(0, HD, 2, dtype=np.float32) / HD))  # [32]
    ifq64 = np.concatenate([inv_freq, inv_freq])  # dim d uses inv_freq[d % 32]
    ang = ifq64[:, None] * positions[None, :].astype(np.float32)  # [64, n]
    cos = np.cos(ang).astype(np.float32)
    sin = np.sin(ang).astype(np.float32)
    sin2 = np.concatenate([sin[:32], -sin[32:]], axis=0)  # sign flip 2nd half
    return np.tile(cos, (2, 1)), np.tile(sin2, (2, 1))


def _dc_block(w):
    """[D, D] -> [128, DC*D] with per-partition layout [dc, k, c]."""
    return np.ascontiguousarray(
        np.asarray(w, dtype=np.float32)
        .reshape(DC, 128, DC, 128)
        .transpose(1, 2, 0, 3)
        .reshape(128, DC * D)
    )


def _bf16(x):
    return np.asarray(x, dtype=np.float32).astype(ml_dtypes.bfloat16)


def prep_in_maps(input_sequence, Wq, Wk, Wv, Wo, bo):
    x = np.asarray(input_sequence, dtype=np.float32)
    wq_b = _bf16(_dc_block(Wq))
    wk_b = _bf16(_dc_block(Wk))
    wv_b = _bf16(np.asarray(Wv))
    wo_b = _bf16(_dc_block(Wo))
    bo_t = np.asarray(bo, dtype=np.float32).reshape(DC, 128).T.copy()

    in_maps = []
    for c in range(8):
        b, t = c // 4, c % 4
        start = t * TC
        lo = start - HALF_W
        xt = np.zeros((D, TH), dtype=np.float32)
        vs = max(0, lo)
        xt[:, vs - lo : TH] = x[b, vs : start + TC, :].T
        cosk_t, sink2_t = _host_tables(np.arange(lo, start + TC))
        # validity of each token chunk's 128 tokens (0 for zero-padded halo)
        tok = np.arange(lo, start + TC)
        valid = (tok >= 0).astype(np.float32).reshape(NTC, 128).T  # [128, NTC]
        vtbl = np.repeat(valid.T[:, None, :], H, axis=1).reshape(NTC * H, 128).T
        in_maps.append(
            {
                "xt": _bf16(xt),
                "wq": wq_b, "wk": wk_b, "wv": wv_b, "wo": wo_b,
                "bo": bo_t,
                "cosk": _bf16(cosk_t), "sink2": _bf16(sink2_t),
                "vtbl": _bf16(np.ascontiguousarray(vtbl)),
                "ident": _bf16(np.eye(128)),
                "perm32": _bf16(
                    np.eye(128, dtype=np.float32)[[p ^ 32 for p in range(128)]]
                ),
            }
        )
    return in_maps


def kernel(input_sequence, Wq, Wk, Wv, Wo, bo):
    nc = _get_nc()
    in_maps = prep_in_maps(input_sequence, Wq, Wk, Wv, Wo, bo)
    res = run_bass_kernel_spmd(nc, in_maps, list(range(8)))
    out = np.empty((B, S, D), dtype=np.float32)
    for c in range(8):
        b, t = c // 4, c % 4
        out[b, t * TC : t * TC + TC, :] = res.results[c]["outT"].T
    return out
